# revision 1
# baseline (speedup 1.0000x reference)
"""GrowableAttention (GQA + RoPE + softmax attention + o_proj) on 8 TRN2 cores.

Sharding: 8 cores = 2 batches x 4 query-blocks of 512 tokens. Each core
computes K/V for its whole batch (redundant across the 4 cores of a batch,
but removes all collectives), attends its 512 queries against all 2048
keys for all 16 heads, and produces a disjoint [512, 2048] row-slice of
the output. Host-side work is only transpose/cast/permute/concat.

v3 schedule (cost-model driven). Phase B (attention) is ACT-bound (the
exp evictions), so the Q projection for heads 4..15 is woven into phase B
as PE filler; phase A is only K proj + V proj + Q quarter 0:
- K proj hi-outer: 4 passes x (4 kv-heads x one 512-col quadrant) = 4
  PSUM banks, coexisting with V proj's 4 banks (no K->V WAR stall). PE
  starts ~2 DMAs in and is fed at DMA line rate.
- Q proj as 2-head half-chains (2 PSUM banks), woven 2 matmuls per
  B-group between scores/attnv; wq tiles stream via a ring pool.
- Softmax denominator: no ones-matmul on PE. exp tiles are tree-summed
  on DVE in bf16 (2x mode, depth-balanced); one gpsimd
  partition_all_reduce per head broadcasts the denominator.
- exp on [128,1024] PSUM tiles (bufs=2) + po double-buffer + Q chains
  = exactly 8 banks; ACT stays saturated through B.
- RoPE in bf16 on DVE (2x) after an ACT PSUM->SBUF bf16 eviction; the
  1/sqrt(128) scale is folded into q-side cos/sin tables, which are
  duplicated onto both partition halves so every DVE op reads
  same-base-partition SBUF operands (hardware requirement).
- x tiles split into query-columns [128,512] (live through B) and
  key-columns [128,1536] (freed before B) to fit SBUF.
- o_proj in 7 PSUM passes (3,3,3,3,2,1,1 tiles, bufs=6 so it starts on
  the earliest-freed banks) keeping the final eviction/DMA tail short.
The key axis is rotated per-core (np.roll) so every core's query block
sits at kt columns 0:512 -> one SPMD program, per-core behavior only
via data.
"""

import math
import sys

sys.path.insert(0, "/opt/trn_rl_repo")

import ml_dtypes
import numpy as np

import concourse.bass as bass
import concourse.bass_isa as bass_isa
from concourse import bacc
import concourse.mybir as mybir
from concourse.bass_utils import run_bass_kernel_spmd
from concourse.tile import TileContext

BF16 = ml_dtypes.bfloat16

NH, NKV, HD = 16, 4, 128
B, S, H = 2, 2048, 2048
T = 512           # queries per core
R = HD // 2       # rope half = 64
HT = H // 128     # 16 hidden k-tiles
KT = S // 128     # 16 key tiles
NCORES = 8
ROPE_THETA = 10000.0

# phase-B kt grouping: 8 groups of 2 kt ([128,1024] PSUM tiles)
BGROUPS = [(k0, 2) for k0 in range(0, 16, 2)]
# phase-C PSUM pass sizes (final passes small -> short drain tail)
CPASSES = [3, 3, 3, 3, 2, 1, 1]

_PROG = None
LAST_RESULTS = None  # BassKernelResults of the most recent run (for test.py)


def _build(upto="C"):
    nc = bacc.Bacc("TRN2", target_bir_lowering=False)
    dt = mybir.dt

    xT = nc.dram_tensor("xT", [H, S], dt.bfloat16, kind="ExternalInput")
    # host-packed, partition-major weights; see _prep_inputs for layouts
    wqP = nc.dram_tensor("wqP", [128, 4 * 8 * 1024], dt.bfloat16,
                         kind="ExternalInput")
    wkP = nc.dram_tensor("wkP", [128, 8 * 1024], dt.bfloat16,
                         kind="ExternalInput")
    wvP = nc.dram_tensor("wvP", [128, 8 * 1024], dt.bfloat16,
                         kind="ExternalInput")
    woT = nc.dram_tensor("woT", [NH * HD, H], dt.bfloat16,
                         kind="ExternalInput")
    # rope tables duplicated onto both partition halves (rows 0:64 ==
    # rows 64:128) so every DVE rope op reads same-base SBUF operands
    cosq = nc.dram_tensor("cosq", [128, T], dt.bfloat16, kind="ExternalInput")
    sinq = nc.dram_tensor("sinq", [128, T], dt.bfloat16, kind="ExternalInput")
    cosk = nc.dram_tensor("cosk", [128, S], dt.bfloat16, kind="ExternalInput")
    sink = nc.dram_tensor("sink", [128, S], dt.bfloat16, kind="ExternalInput")
    out_d = nc.dram_tensor("out", [T, H], dt.float32, kind="ExternalOutput")

    Exp = mybir.ActivationFunctionType.Exp

    with TileContext(nc) as tc:
        with (
            tc.tile_pool(name="persist", bufs=1) as pp,
            tc.tile_pool(name="xq", bufs=1) as xqp,
            tc.tile_pool(name="wqs", bufs=12) as wqp,
            tc.tile_pool(name="evt", bufs=5) as evp,
            tc.tile_pool(name="rtmp", bufs=4) as rt,
        ):
            kts = [pp.tile([128, S], dt.bfloat16, tag=f"k{i}", name=f"k{i}")
                   for i in range(NKV)]
            qts = [pp.tile([128, T], dt.bfloat16, tag=f"q{i}", name=f"q{i}")
                   for i in range(NH)]
            vts = [pp.tile([128, NKV * HD], dt.bfloat16, tag=f"v{i}",
                           name=f"v{i}") for i in range(KT)]
            aot = [pp.tile([128, T], dt.bfloat16, tag=f"ao{i}", name=f"ao{i}")
                   for i in range(NH)]
            cq = sq = ck = sk = None
            xq = [None] * HT

            def rope_evict(ps, cos_t, sin_t, dst, col0, on_act=True):
                # dst[0:64]  = ps[0:64]*cos - ps[64:128]*sin
                # dst[64:128]= ps[64:128]*cos + ps[0:64]*sin
                # PSUM->SBUF bf16 evict on ACT, then rope on DVE all in
                # bf16 2x mode.  Two SBUF inputs of a DVE op must share a
                # base partition: the tables are duplicated on both
                # partition halves and each product is WRITTEN to the
                # half its consumer reads from (out base may differ).
                tmp = evp.tile([128, 512], dt.bfloat16, tag="ev", name="ev")
                if on_act:
                    nc.scalar.copy(out=tmp, in_=ps)
                else:
                    # phase B: ACT is the exp bottleneck; evict on DVE
                    nc.vector.tensor_copy(out=tmp, in_=ps)
                w1 = rt.tile([128, 512], dt.bfloat16, tag="r1", name="r1")
                w2 = rt.tile([128, 512], dt.bfloat16, tag="r2", name="r2")
                # products for dst top, staged at base 64
                nc.vector.tensor_mul(out=w1[R:128, :], in0=tmp[0:R, :],
                                     in1=cos_t[0:R, :])
                nc.vector.tensor_mul(out=w2[R:128, :], in0=tmp[R:128, :],
                                     in1=sin_t[R:128, :])
                nc.vector.tensor_sub(
                    out=dst[0:R, col0:col0 + 512], in0=w1[R:128, :],
                    in1=w2[R:128, :])
                # products for dst bottom, staged at base 0
                nc.vector.tensor_mul(out=w1[0:R, :], in0=tmp[R:128, :],
                                     in1=cos_t[R:128, :])
                nc.vector.tensor_mul(out=w2[0:R, :], in0=tmp[0:R, :],
                                     in1=sin_t[0:R, :])
                nc.vector.tensor_add(
                    out=dst[R:128, col0:col0 + 512], in0=w1[0:R, :],
                    in1=w2[0:R, :])

            # ============ phase A: K proj, V proj (+ x/w DMAs) ============
            with (
                tc.tile_pool(name="xk", bufs=1) as xkp,
                tc.tile_pool(name="wkp", bufs=1) as wkp,
                tc.tile_pool(name="wvp", bufs=1) as wvp,
                tc.tile_pool(name="psK", bufs=1, space="PSUM") as pk,
                tc.tile_pool(name="psV", bufs=4, space="PSUM") as pV,
            ):
                # --- DMAs, interleaved so the PE starts right after the
                # first (wk, xq) pair lands.  x columns are split:
                # xq [128,512] = this core's query block (lives through
                # B); xk [128,1536] = remaining key columns (freed after
                # V proj).  K pass p reads x cols p*512:(p+1)*512. ---
                xk = []
                wk_t = []
                for hp in range(HT // 2):
                    w = wkp.tile([128, 1024], dt.bfloat16,
                                 tag=f"wk{hp}", name=f"wk{hp}")
                    nc.sync.dma_start(
                        out=w, in_=wkP[:, hp * 1024:(hp + 1) * 1024])
                    wk_t.append(w)
                    for hj in range(2):
                        hi = 2 * hp + hj
                        x = xqp.tile([128, T], dt.bfloat16, tag=f"xq{hi}",
                                     name=f"xq{hi}")
                        if hi == 0:
                            nc.scalar.dma_start(
                                out=x,
                                in_=xT[hi * 128:(hi + 1) * 128, 0:T])
                        else:
                            nc.sync.dma_start(
                                out=x,
                                in_=xT[hi * 128:(hi + 1) * 128, 0:T])
                        xq[hi] = x
                    if hp == 0:
                        # k-side rope table 1: needed from the first K
                        # evict (~14us); interleaved behind the first
                        # DMA pairs so the K feed is not starved
                        ck = pp.tile([128, S], dt.bfloat16, tag="ck",
                                     name="ck")
                        nc.sync.dma_start(out=ck, in_=cosk[:, :])
                    if hp == 1:
                        sk = pp.tile([128, S], dt.bfloat16, tag="sk",
                                     name="sk")
                        nc.sync.dma_start(out=sk, in_=sink[:, :])
                # key-column x (pass 2 chunk first, then passes 3-4 + wv)
                for hp in range(HT // 2):
                    for hj in range(2):
                        hi = 2 * hp + hj
                        x = xkp.tile([128, 3 * 512], dt.bfloat16,
                                     tag=f"xk{hi}", name=f"xk{hi}")
                        nc.sync.dma_start(
                            out=x[:, 0:512],
                            in_=xT[hi * 128:(hi + 1) * 128, 512:1024])
                        xk.append(x)
                wv_t = []
                for hp in range(HT // 2):
                    for hj in range(2):
                        hi = 2 * hp + hj
                        nc.sync.dma_start(
                            out=xk[hi][:, 512:1536],
                            in_=xT[hi * 128:(hi + 1) * 128, 1024:2048])
                    w = wvp.tile([128, 1024], dt.bfloat16,
                                 tag=f"wv{hp}", name=f"wv{hp}")
                    nc.sync.dma_start(
                        out=w, in_=wvP[:, hp * 1024:(hp + 1) * 1024])
                    wv_t.append(w)
                # q-side rope tables: first needed at the Q0 evict
                cq = pp.tile([128, T], dt.bfloat16, tag="cq", name="cq")
                nc.sync.dma_start(out=cq, in_=cosq[:, :])
                sq = pp.tile([128, T], dt.bfloat16, tag="sq", name="sq")
                nc.sync.dma_start(out=sq, in_=sinq[:, :])

                def xcols(hi, c0):
                    # x columns [c0, c0+512) for row-block hi
                    if c0 < 512:
                        return xq[hi][:, c0:c0 + 512]
                    return xk[hi][:, c0 - 512:c0]

                # --- K projection, hi-outer: 4 passes x 4 kv-heads ---
                for p in range(4):
                    psk = [pk.tile([128, 512], dt.float32,
                                   tag=f"pk{kh}", name=f"pk{kh}")
                           for kh in range(NKV)]
                    for hi in range(HT):
                        w = wk_t[hi // 2]
                        for kh in range(NKV):
                            nc.tensor.matmul(
                                out=psk[kh],
                                lhsT=w[:, (hi % 2) * 512 + kh * HD:
                                       (hi % 2) * 512 + (kh + 1) * HD],
                                rhs=xcols(hi, p * 512),
                                start=(hi == 0), stop=(hi == HT - 1))
                    for kh in range(NKV):
                        c0 = p * 512
                        rope_evict(psk[kh], ck[:, c0:c0 + 512],
                                   sk[:, c0:c0 + 512], kts[kh], c0)

                # --- V projection (kt-outer, hi-chains), own 4 banks ---
                for kt in range(KT):
                    ps = pV.tile([128, NKV * HD], dt.float32, tag="pv",
                                 name="pv")
                    for hi in range(HT):
                        nc.tensor.matmul(
                            out=ps,
                            lhsT=xcols(hi, (kt // 4) * 512)[
                                :, (kt % 4) * 128:(kt % 4 + 1) * 128],
                            rhs=wv_t[hi // 2][:, (hi % 2) * 512:
                                              (hi % 2 + 1) * 512],
                            start=(hi == 0), stop=(hi == HT - 1))
                    nc.scalar.copy(out=vts[kt], in_=ps)

            # ======== Q projection machinery (quarter 0 in phase A,
            # quarters 1-3 woven into phase B as PE filler) ========
            with (
                tc.tile_pool(name="ets", bufs=5) as es,
                tc.tile_pool(name="tree", bufs=1) as tb,
                tc.tile_pool(name="smallf", bufs=2) as sf,
                tc.tile_pool(name="wos", bufs=16) as wop,
                tc.tile_pool(name="outp", bufs=4) as op_,
              ):
              with tc.tile_pool(name="psQ", bufs=1, space="PSUM") as pQ:
                wq_tiles = {}
                for qq in range(4):
                    for hp in range(HT // 2):
                        w = wqp.tile([128, 1024], dt.bfloat16, tag="wq",
                                     name="wq")
                        nc.sync.dma_start(
                            out=w,
                            in_=wqP[:, qq * 8192 + hp * 1024:
                                    qq * 8192 + (hp + 1) * 1024])
                        wq_tiles[(qq, hp)] = w

                pq_t = [pQ.tile([128, T], dt.float32, tag=f"pq{j}",
                                name=f"pq{j}") for j in range(2)]
                wo_t = {}    # filled at phase-B start
                pqc = []     # early o_proj tiles (out blocks 0 and 1)

                def q_actions():
                    # one yielded action = one hi-step (2 matmuls) of a
                    # 2-head half-chain; evicts emitted inline after the
                    # chain's last step.  After the quarters, yields early
                    # o_proj chain steps (heads 0..11, output tiles 0-1)
                    # into the freed pQ banks — PE filler for the
                    # ACT-bound B heads 12-15.
                    for qq in range(4):
                        for half in range(2):
                            for hi in range(HT):
                                for j in range(2):
                                    h4 = 2 * half + j
                                    nc.tensor.matmul(
                                        out=pq_t[j],
                                        lhsT=wq_tiles[(qq, hi // 2)][
                                            :, (hi % 2) * 512 + h4 * HD:
                                            (hi % 2) * 512 + (h4 + 1) * HD],
                                        rhs=xq[hi],
                                        start=(hi == 0), stop=(hi == HT - 1))
                                if hi == HT - 1:
                                    for j in range(2):
                                        rope_evict(
                                            pq_t[j], cq, sq,
                                            qts[qq * 4 + 2 * half + j], 0,
                                            on_act=(qq == 0))
                                yield None

                q_iter = q_actions()

                def pull_q(n):
                    for _ in range(n):
                        next(q_iter, None)

                # phase A tail: Q quarter 0 (heads 0-3)
                pull_q(2 * HT)

                # ============ phase B: attention (+ Q weave) ============
                if upto in ("B", "C"):
                    with (
                        tc.tile_pool(name="psS", bufs=2, space="PSUM")
                        as pSm,
                        tc.tile_pool(name="psO", bufs=2, space="PSUM")
                        as pO,
                    ):
                        # prefetch wo (first output-column half) during B
                        for h in range(NH):
                            w = wop.tile([128, H // 2], dt.bfloat16,
                                         tag="wo", name="wo")
                            nc.sync.dma_start(
                                out=w,
                                in_=woT[h * 128:(h + 1) * 128, 0:H // 2])
                            wo_t[(0, h)] = w

                        for h in range(NH):
                            kh = h // (NH // NKV)
                            po = pO.tile([128, T], dt.float32, tag="po",
                                         name="po")
                            partials = []
                            prev = None  # (et, k0) awaiting attnv
                            for gi, (k0, nkt) in enumerate(BGROUPS):
                                ps = pSm.tile([128, nkt * 512], dt.float32,
                                              tag="ps2", name="ps2")
                                for j in range(nkt):
                                    nc.tensor.matmul(
                                        out=ps[:, j * 512:(j + 1) * 512],
                                        lhsT=kts[kh][:, (k0 + j) * 128:
                                                     (k0 + j + 1) * 128],
                                        rhs=qts[h], start=True, stop=True)
                                et = es.tile([128, nkt * 512], dt.bfloat16,
                                             tag="et2", name="et2")
                                nc.scalar.activation(out=et, in_=ps,
                                                     func=Exp)
                                pull_q(1)  # filler hides exp->attnv sem
                                if prev is not None:
                                    pet, pk0 = prev
                                    for j in range(nkt):
                                        nc.tensor.matmul(
                                            out=po,
                                            lhsT=vts[pk0 + j][
                                                :, kh * HD:(kh + 1) * HD],
                                            rhs=pet[:, j * 512:
                                                    (j + 1) * 512],
                                            start=(gi == 1 and j == 0),
                                            stop=False)
                                pt = tb.tile([128, T], dt.bfloat16,
                                             tag=f"pt{gi}", name=f"pt{gi}")
                                nc.vector.tensor_add(
                                    out=pt, in0=et[:, 0:512],
                                    in1=et[:, 512:1024])
                                partials.append(pt)
                                prev = (et, k0)
                            pet, pk0 = prev
                            for j in range(2):
                                nc.tensor.matmul(
                                    out=po,
                                    lhsT=vts[pk0 + j][:, kh * HD:
                                                      (kh + 1) * HD],
                                    rhs=pet[:, j * 512:(j + 1) * 512],
                                    start=False, stop=(j == 1))
                            # combine 8 partials (depth-balanced, final
                            # add in fp32)
                            nxt = []
                            for i in range(4):
                                m = tb.tile([128, T], dt.bfloat16,
                                            tag=f"m{i}", name=f"m{i}")
                                nc.vector.tensor_add(out=m,
                                                     in0=partials[2 * i],
                                                     in1=partials[2 * i + 1])
                                nxt.append(m)
                            m4 = tb.tile([128, T], dt.bfloat16, tag="m4",
                                         name="m4")
                            nc.vector.tensor_add(out=m4, in0=nxt[0],
                                                 in1=nxt[1])
                            m5 = tb.tile([128, T], dt.bfloat16, tag="m5",
                                         name="m5")
                            nc.vector.tensor_add(out=m5, in0=nxt[2],
                                                 in1=nxt[3])
                            dsum = sf.tile([128, T], dt.float32, tag="ds",
                                           name="ds")
                            nc.vector.tensor_add(out=dsum, in0=m4, in1=m5)
                            # partition reduce on Pool: every partition
                            # gets the full per-query denominator
                            dall = sf.tile([128, T], dt.float32, tag="da",
                                           name="da")
                            nc.gpsimd.partition_all_reduce(
                                dall, dsum, channels=128,
                                reduce_op=bass_isa.ReduceOp.add)
                            rc = sf.tile([128, T], dt.float32, tag="rc",
                                         name="rc")
                            nc.vector.reciprocal(out=rc, in_=dall)
                            nc.vector.tensor_mul(out=aot[h], in0=po,
                                                 in1=rc)

              # ============ phase C: o_proj ============
              if upto == "C":
                with tc.tile_pool(name="psC", bufs=6,
                                  space="PSUM") as pC:
                  tile0 = 0
                  for cp, npc in enumerate(CPASSES):
                      if cp == 1:
                          # second wo half: DMAs pace themselves via
                          # the ring pool's WAR on the first half
                          for hh in range(NH):
                              w = wop.tile([128, H // 2], dt.bfloat16,
                                           tag="wo", name="wo")
                              nc.sync.dma_start(
                                  out=w,
                                  in_=woT[hh * 128:(hh + 1) * 128,
                                          H // 2:H])
                              wo_t[(1, hh)] = w
                      # tile index t in [tile0, tile0+npc): output
                      # block (mh, tm, nsi), t = mh*8 + tm*2 + nsi
                      pcs = [pC.tile([128, 512], dt.float32,
                                     tag="pc", name="pc")
                             for _ in range(npc)]
                      for h in range(NH):
                          for ti in range(npc):
                              t = tile0 + ti
                              mh, tm, nsi = t // 8, (t % 8) // 2, t % 2
                              nc.tensor.matmul(
                                  out=pcs[ti],
                                  lhsT=aot[h][:, tm * 128:
                                              (tm + 1) * 128],
                                  rhs=wo_t[(mh, h)][
                                      :, nsi * 512:(nsi + 1) * 512],
                                  start=(h == 0), stop=(h == NH - 1))
                      for ti in range(npc):
                          t = tile0 + ti
                          mh, tm, nsi = t // 8, (t % 8) // 2, t % 2
                          ot = op_.tile([128, 512], dt.float32,
                                        tag="ot", name="ot")
                          # ACT is idle in phase C; DVE copy -> ACT copy
                          nc.scalar.copy(out=ot, in_=pcs[ti])
                          nc.sync.dma_start(
                              out=out_d[tm * 128:(tm + 1) * 128,
                                        mh * 1024 + nsi * 512:
                                        mh * 1024 + (nsi + 1) * 512],
                              in_=ot)
                      tile0 += npc
    nc.finalize()
    return nc


def _pack_pairs(wT, ncols):
    """[2048, ncols] -> [128, 8 * 2 * ncols]: hi-pair-major, partition-major.
    out[p, hp*2*ncols + b*ncols + j] = wT[(2*hp + b)*128 + p, j]."""
    return np.ascontiguousarray(
        wT.reshape(8, 2, 128, ncols).transpose(2, 0, 1, 3).reshape(128, -1))


def _prep_inputs(hidden_states, Wq, Wk, Wv, Wo):
    inv = 1.0 / (ROPE_THETA ** (np.arange(0, HD, 2, dtype=np.float32) / HD))
    pos = np.arange(S, dtype=np.float32)
    fr = inv[:, None] * pos[None, :]            # [R, S]
    cosk = np.cos(fr).astype(np.float32)
    sink = np.sin(fr).astype(np.float32)
    sc = np.float32(1.0 / math.sqrt(HD))

    wqT = np.ascontiguousarray(np.asarray(Wq).T).astype(BF16)
    wkT = np.ascontiguousarray(np.asarray(Wk).T).astype(BF16)
    wvT = np.ascontiguousarray(np.asarray(Wv).T).astype(BF16)
    woT = np.ascontiguousarray(np.asarray(Wo).T).astype(BF16)
    # wq: per-quarter pair packing:
    # wqP[p, qq*8192 + hp*1024 + b*512 + j] = wqT[(2hp+b)*128+p, qq*512+j]
    wqP = np.ascontiguousarray(
        wqT.reshape(8, 2, 128, 4, 512).transpose(2, 3, 0, 1, 4)
        .reshape(128, -1))
    wkP = _pack_pairs(wkT, 512)
    wvP = _pack_pairs(wvT, 512)
    hs = np.asarray(hidden_states)

    in_maps = []
    for c in range(NCORES):
        b, qb = divmod(c, 4)
        perm = np.roll(np.arange(S), -qb * T)
        xTp = np.ascontiguousarray(hs[b].T[:, perm]).astype(BF16)
        in_maps.append({
            "xT": xTp,
            "wqP": wqP, "wkP": wkP, "wvP": wvP, "woT": woT,
            "cosq": np.tile((cosk[:, qb * T:(qb + 1) * T] * sc)
                            .astype(BF16), (2, 1)),
            "sinq": np.tile((sink[:, qb * T:(qb + 1) * T] * sc)
                            .astype(BF16), (2, 1)),
            "cosk": np.tile(np.ascontiguousarray(cosk[:, perm])
                            .astype(BF16), (2, 1)),
            "sink": np.tile(np.ascontiguousarray(sink[:, perm])
                            .astype(BF16), (2, 1)),
        })
    return in_maps


def kernel(hidden_states, Wq, Wk, Wv, Wo, _trace=False):
    global _PROG, LAST_RESULTS
    if _PROG is None:
        _PROG = _build()
    in_maps = _prep_inputs(hidden_states, Wq, Wk, Wv, Wo)
    res = run_bass_kernel_spmd(
        _PROG, in_maps, core_ids=list(range(NCORES)), trace=_trace)
    LAST_RESULTS = res
    full = np.empty((B, S, H), np.float32)
    for c in range(NCORES):
        b, qb = divmod(c, 4)
        full[b, qb * T:(qb + 1) * T, :] = res.results[c]["out"]
    return full

